# revision 22
# baseline (speedup 1.0000x reference)
"""Trainium2 Bass kernel for masked+modulated multi-head attention.

Reference computation (per batch b):
    xn = LayerNorm(x) * gamma + beta
    q,k,v = split(xn @ w_qkv); per head: dots = (q k^T) * scale
    dots = where(labels==0 on key, -1e9, dots) * (1 + con)
    attn = softmax(dots, axis=key);  out = (attn @ v reshaped) @ w_out
    returns (out, attn)

Sharding: 8 cores = 2 batches x 4 head-groups (3 heads each).
Each core computes, entirely on device, for its (b, 3 heads):
  - LayerNorm of x[b] (gamma/beta folded into w_qkv / bias on host)
  - q^T,k^T,v per head with an augmented contraction row that applies the
    key padding mask inside the dots matmul (q row 64 = 1, k row 64 = mask)
  - scores TRANSPOSED: S^T[k,q] tiles so that softmax numerators p=exp(s)
    feed the attn@v matmul directly (no on-chip transpose of the 50MB attn)
  - softmax denominators via ones-columns appended to v: PSUM rows 64..127
    of the attn@v output all hold sum_k exp(s), so the reciprocal runs
    64 lanes wide and no cross-partition broadcast is needed
  - attn^T (written k-major; host views it back) and the w_out partial sum
Matmul operands are bf16 (f32 accumulation in PSUM); softmax s stays f32.
k-tiles are processed in pairs so DVE/ACT ops run at [128,1024] granularity.
Host: shards inputs, transposes con once, sums the 4 out-partials per batch,
and transposes attn back to [b, h, q, k] (numpy view manipulation only).
"""

import numpy as np
import ml_dtypes

import concourse.bass as bass
import concourse.bacc as baccmod
import concourse.mybir as mybir
import concourse.tile as tile
from concourse.bass_utils import run_bass_kernel_spmd
from concourse.masks import make_identity

HEADS = 12
DH = 64
DIM = 768
N = 2048
B = 2
NCORES = 8
HPC = 3          # heads per core
CPB = 4          # cores per batch
SCALE = DH ** -0.5
EPS = 1e-5
NEG = -1e9

F32 = mybir.dt.float32
BF16 = mybir.dt.bfloat16
AF = mybir.ActivationFunctionType
ALU = mybir.AluOpType
BFNP = ml_dtypes.bfloat16

NT = N // 128            # 16 k token tiles
NP = NT // 2             # 8 k tile pairs
NQ = N // 512            # 4 query chunks
KF = DIM // 128          # 6 feature k-tiles

# which engine runs the attn normalize+store multiply, per k-pair index
NORM_ENGINE = ["gpsimd", "vector"] * (NP // 2)


def build_nc() -> bass.Bass:
    nc = baccmod.Bacc("TRN2")

    xb = nc.declare_dram_parameter("xb", [NT, 128, DIM], F32, isOutput=False)
    conT = nc.declare_dram_parameter("conT", [N, N], F32, isOutput=False)
    wqkv = nc.declare_dram_parameter("wqkv", [KF, 128, 576], BF16, isOutput=False)
    bqkv = nc.declare_dram_parameter("bqkv", [5, 128], F32, isOutput=False)
    maskrow = nc.declare_dram_parameter("maskrow", [1, N], BF16, isOutput=False)
    wo01 = nc.declare_dram_parameter("wo01", [128, DIM], BF16, isOutput=False)
    wo2 = nc.declare_dram_parameter("wo2", [64, DIM], BF16, isOutput=False)
    attn_t = nc.declare_dram_parameter("attn_t", [HPC, N, N], F32, isOutput=True)
    out_part = nc.declare_dram_parameter("out_part", [NT, 128, DIM], F32, isOutput=True)

    with tile.TileContext(nc) as tc:
        with (
            tc.tile_pool(name="singles", bufs=1) as singles,
            tc.tile_pool(name="persist", bufs=1) as persist,
        ):
            identb = singles.tile([128, 128], BF16, tag="identb")
            make_identity(nc, identb)
            eps_sb = singles.tile([128, 1], F32, tag="eps")
            nc.vector.memset(eps_sb, EPS)
            wo01_sb = singles.tile([128, DIM], BF16, tag="wo01")
            nc.sync.dma_start(out=wo01_sb, in_=wo01[:])
            wo2_sb = singles.tile([64, DIM], BF16, tag="wo2")
            nc.sync.dma_start(out=wo2_sb, in_=wo2[:])

            # persistent per-head tensors (all bf16 matmul operands)
            qT = [persist.tile([65, N], BF16, tag=f"qT{h}", name=f"qT{h}")
                  for h in range(HPC)]
            kT = [persist.tile([65, N], BF16, tag=f"kT{h}", name=f"kT{h}")
                  for h in range(HPC)]
            # v2 cols 0..63 = v^T; cols 64..127 all ones (softmax denominators)
            v2 = [persist.tile([128, NT, 128], BF16, tag=f"v2{h}", name=f"v2{h}")
                  for h in range(HPC)]
            O01 = persist.tile([128, N], BF16, tag="O01")
            O2 = persist.tile([64, N], BF16, tag="O2")

            for h in range(HPC):
                nc.vector.memset(qT[h][64:65, :], 1.0)
                nc.sync.dma_start(out=kT[h][64:65, :], in_=maskrow[:])
                nc.vector.memset(v2[h], 1.0)  # cols 0..63 overwritten later

            # conp pool created BEFORE phase A pools so its SBUF range is
            # disjoint: the first query chunks' con loads prefetch during
            # phase A instead of waiting for phase A buffers to free
            _pconp_cm = tc.tile_pool(name="conp", bufs=2)
            pconp = _pconp_cm.__enter__()

            # ---------------- Phase A: LN + transpose + QKV ----------------
            with (
                tc.tile_pool(name="pa", bufs=3) as pa,
                tc.tile_pool(name="xt", bufs=1) as xtp,
                tc.tile_pool(name="pa_ps", bufs=2, space="PSUM") as pap,
            ):
                w_sb = [xtp.tile([128, 576], BF16, tag=f"w{i}", name=f"w{i}")
                        for i in range(KF)]
                for i in range(KF):
                    nc.sync.dma_start(out=w_sb[i], in_=wqkv[i])
                bias_sb = xtp.tile([128, 5], F32, tag="bias")
                for g in range(5):
                    nc.sync.dma_start(
                        out=bias_sb[:, g : g + 1], in_=bqkv[g].unsqueeze(1)
                    )
                xhatT = [xtp.tile([128, N], BF16, tag=f"xt{i}", name=f"xt{i}")
                         for i in range(KF)]
                vt_tmp = [xtp.tile([64, N], BF16, tag=f"vt{h}", name=f"vt{h}")
                          for h in range(HPC)]

                for tt in range(NT):
                    xtile = pa.tile([128, DIM], F32, tag="x")
                    nc.sync.dma_start(out=xtile, in_=xb[tt])
                    stats = pa.tile([128, 2, 6], F32, tag="st")
                    nc.vector.bn_stats(out=stats[:, 0, :], in_=xtile[:, 0:512])
                    nc.vector.bn_stats(out=stats[:, 1, :], in_=xtile[:, 512:768])
                    mv = pa.tile([128, 2], F32, tag="mv")
                    nc.vector.bn_aggr(out=mv, in_=stats)
                    rstd = pa.tile([128, 1], F32, tag="rstd")
                    nc.scalar.activation(
                        out=rstd, in_=mv[:, 1:2], func=AF.Sqrt, bias=eps_sb
                    )
                    nc.vector.reciprocal(out=rstd, in_=rstd)
                    nmu = pa.tile([128, 1], F32, tag="nmu")
                    nc.vector.scalar_tensor_tensor(
                        out=nmu, in0=mv[:, 0:1], scalar=-1.0, in1=rstd,
                        op0=ALU.mult, op1=ALU.mult,
                    )
                    xhat = pa.tile([128, DIM], BF16, tag="xh")
                    nc.scalar.activation(
                        out=xhat, in_=xtile, func=AF.Identity, bias=nmu, scale=rstd
                    )
                    for ft in range(KF):
                        pst = pap.tile([128, 128], BF16, tag="tr")
                        nc.tensor.transpose(
                            pst, xhat[:, ft * 128 : (ft + 1) * 128], identb
                        )
                        nc.vector.tensor_copy(
                            out=xhatT[ft][:, tt * 128 : (tt + 1) * 128], in_=pst
                        )

                # QKV projection, head-major column order:
                # [q0 k0 v0 q1 k1 v1 q2 k2 v2] (64 each) so head 0's
                # attention can start while later heads still project
                groups = [(0, 128), (128, 256), (256, 384), (384, 512), (512, 576)]
                destmap = [
                    [qT[0], kT[0]],
                    [vt_tmp[0], qT[1]],
                    [kT[1], vt_tmp[1]],
                    [qT[2], kT[2]],
                    [vt_tmp[2]],
                ]
                vdone = {1: 0, 2: 1, 4: 2}  # group -> head whose v completes

                def v_transpose(h):
                    for kt in range(NT):
                        psv = pap.tile([128, 64], BF16, tag="trv", name="psv")
                        nc.tensor.transpose(
                            psv,
                            vt_tmp[h][0:64, kt * 128 : (kt + 1) * 128],
                            identb[0:64, 0:64],
                        )
                        nc.scalar.copy(out=v2[h][:, kt, 0:64], in_=psv)

                for g, (c0, c1) in enumerate(groups):
                    m = c1 - c0
                    for t4 in range(NQ):
                        ps = pap.tile([128, 512], F32, tag="mm")
                        for kc in range(KF):
                            nc.tensor.matmul(
                                ps[:m],
                                w_sb[kc][:, c0:c1],
                                xhatT[kc][:, t4 * 512 : (t4 + 1) * 512],
                                start=(kc == 0),
                                stop=(kc == KF - 1),
                            )
                        for half, dest in enumerate(destmap[g]):
                            nc.scalar.activation(
                                out=dest[0:64, t4 * 512 : (t4 + 1) * 512],
                                in_=ps[half * 64 : half * 64 + 64],
                                func=AF.Identity,
                                bias=bias_sb[half * 64 : half * 64 + 64, g : g + 1],
                            )
                    if g in vdone:
                        v_transpose(vdone[g])

            # ---------------- Phase B: attention ----------------
            with (
                tc.tile_pool(name="ps_s", bufs=2) as pss,
                tc.tile_pool(name="pp", bufs=2) as pp,
                tc.tile_pool(name="pat", bufs=2) as pat,
                tc.tile_pool(name="psmall", bufs=2) as psmall,
                tc.tile_pool(name="pb_s", bufs=3, space="PSUM") as pbs,
                tc.tile_pool(name="pb_o", bufs=2, space="PSUM") as pbo,
            ):
                # software pipeline across (qc, h) blocks: block i's attn
                # normalize+store multiplies are interleaved into block i+1's
                # score loop so DVE never runs a long norm burst that starves
                # the next head's psS slots
                prev = None  # (pk_pairs, rb, atb, h, q0)

                def norm_step(kp):
                    pk_pairs, rb, atb, ph, pq0 = prev
                    nc.vector.tensor_tensor(
                        atb[:, 2 * kp : 2 * kp + 2, :],
                        pk_pairs[kp],
                        rb.unsqueeze(1).broadcast_to([128, 2, 512]),
                        ALU.mult,
                    )

                def norm_flush():
                    _, _, atb, ph, pq0 = prev
                    nc.gpsimd.dma_start(
                        out=attn_t[ph].rearrange(
                            "(kt p) q -> p kt q", p=128
                        )[:, :, pq0 : pq0 + 512],
                        in_=atb,
                    )

                for qc in range(NQ):
                    q0 = qc * 512
                    conp = pconp.tile([128, NT, 512], F32, tag="conp")
                    for kt in range(NT):
                        nc.sync.dma_start(
                            out=conp[:, kt, :],
                            in_=conT[kt * 128 : (kt + 1) * 128, q0 : q0 + 512],
                        )
                    for kp in range(NP):
                        nc.vector.tensor_scalar_add(
                            conp[:, 2 * kp : 2 * kp + 2, :],
                            conp[:, 2 * kp : 2 * kp + 2, :],
                            1.0,
                        )
                    for h in range(HPC):
                        psO = pbo.tile([128, 512], F32, tag="O")
                        pk_pairs = []
                        for kp in range(NP):
                            psSp = pbs.tile([128, 2, 512], F32, tag="S")
                            for j in range(2):
                                kt = 2 * kp + j
                                nc.tensor.matmul(
                                    psSp[:, j, :],
                                    kT[h][:, kt * 128 : (kt + 1) * 128],
                                    qT[h][:, q0 : q0 + 512],
                                    start=True,
                                    stop=True,
                                )
                            sp = pss.tile([128, 2, 512], F32, tag="s")
                            nc.vector.tensor_mul(
                                sp, psSp, conp[:, 2 * kp : 2 * kp + 2, :]
                            )
                            pkp = pp.tile([128, 2, 512], BF16, tag=f"p{kp}",
                                          name=f"p{kp}")
                            nc.scalar.activation(out=pkp, in_=sp, func=AF.Exp)
                            for j in range(2):
                                kt = 2 * kp + j
                                nc.tensor.matmul(
                                    psO,
                                    v2[h][:, kt, :],
                                    pkp[:, j, :],
                                    start=(kt == 0),
                                    stop=(kt == NT - 1),
                                )
                            pk_pairs.append(pkp)
                            if prev is not None:
                                norm_step(kp)
                        if prev is not None:
                            norm_flush()
                        # row 64 of psO holds sum_k exp(s) per q
                        sums = psmall.tile([1, 512], F32, tag="sums")
                        nc.vector.tensor_copy(out=sums, in_=psO[64:65, :])
                        recip = psmall.tile([1, 512], F32, tag="recip")
                        nc.vector.reciprocal_approx_fast(out=recip, in_=sums)
                        recipb = psmall.tile([1, 512], BF16, tag="recipb")
                        nc.vector.tensor_copy(out=recipb, in_=recip)
                        rb = psmall.tile([128, 512], BF16, tag="rb")
                        nc.gpsimd.partition_broadcast(rb, recipb)
                        atb = pat.tile([128, NT, 512], BF16, tag="at")
                        if h < 2:
                            dest = O01[h * 64 : (h + 1) * 64, q0 : q0 + 512]
                        else:
                            dest = O2[:, q0 : q0 + 512]
                        nc.vector.tensor_mul(dest, psO[0:64, :], rb[0:64, :])
                        prev = (pk_pairs, rb, atb, h, q0)
                # epilogue: normalize+store the final block
                for kp in range(NP):
                    norm_step(kp)
                norm_flush()

            _pconp_cm.__exit__(None, None, None)

            # ---------------- Phase C: output projection ----------------
            with (
                tc.tile_pool(name="pc", bufs=2) as pc,
                tc.tile_pool(name="pc_ps", bufs=2, space="PSUM") as pcp,
            ):
                for tt in range(NT):
                    outp = pc.tile([128, DIM], F32, tag="op")
                    for n0, n1 in [(0, 512), (512, 768)]:
                        pso = pcp.tile([128, 512], F32, tag="mm2")
                        nc.tensor.matmul(
                            pso[:, : n1 - n0],
                            O01[:, tt * 128 : (tt + 1) * 128],
                            wo01_sb[:, n0:n1],
                            start=True,
                            stop=False,
                        )
                        nc.tensor.matmul(
                            pso[:, : n1 - n0],
                            O2[:, tt * 128 : (tt + 1) * 128],
                            wo2_sb[:, n0:n1],
                            start=False,
                            stop=True,
                        )
                        nc.scalar.copy(out=outp[:, n0:n1], in_=pso[:, : n1 - n0])
                    nc.sync.dma_start(out=out_part[tt], in_=outp)

    nc.compile()
    return nc


_NC_CACHE = None


def get_nc() -> bass.Bass:
    global _NC_CACHE
    if _NC_CACHE is None:
        _NC_CACHE = build_nc()
    return _NC_CACHE


def make_in_maps(x, labels, con, gamma, beta, w_qkv, w_out):
    """Host-side sharding: returns list of 8 per-core input dicts."""
    x = np.asarray(x, dtype=np.float32)
    labels = np.asarray(labels)
    con = np.asarray(con, dtype=np.float32)
    gamma = np.asarray(gamma, dtype=np.float32)
    beta = np.asarray(beta, dtype=np.float32)
    w_qkv = np.asarray(w_qkv, dtype=np.float32)
    w_out = np.asarray(w_out, dtype=np.float32)

    # fold gamma into the qkv weights, and q's 1/sqrt(d) scale into wq/bq
    wq = (gamma[:, None] * w_qkv[:, :DIM]) * SCALE
    wk = gamma[:, None] * w_qkv[:, DIM : 2 * DIM]
    wv = gamma[:, None] * w_qkv[:, 2 * DIM :]
    bq = (beta @ w_qkv[:, :DIM]) * SCALE
    bk = beta @ w_qkv[:, DIM : 2 * DIM]
    bv = beta @ w_qkv[:, 2 * DIM :]

    in_maps = []
    for c in range(NCORES):
        b = c // CPB
        h0 = HPC * (c % CPB)
        cols = []
        bcols = []
        for h in range(HPC):
            sl = slice((h0 + h) * DH, (h0 + h + 1) * DH)
            cols += [wq[:, sl], wk[:, sl], wv[:, sl]]
            bcols += [bq[sl], bk[sl], bv[sl]]
        wqkv_c = np.concatenate(cols, axis=1)  # [768, 576]
        bqkv_c = np.concatenate(bcols + [np.zeros(64, np.float32)]).reshape(5, 128)
        wo_c = w_out[h0 * DH : (h0 + HPC) * DH]  # [192, 768]
        in_maps.append(
            {
                "xb": np.ascontiguousarray(x[b].reshape(NT, 128, DIM)),
                "conT": np.ascontiguousarray(con[b].T),
                "wqkv": np.ascontiguousarray(wqkv_c.reshape(KF, 128, 576)).astype(BFNP),
                "bqkv": np.ascontiguousarray(bqkv_c),
                "maskrow": np.where(labels[b] == 0, np.float32(NEG), np.float32(0.0))
                .astype(BFNP)
                .reshape(1, N),
                "wo01": np.ascontiguousarray(wo_c[:128]).astype(BFNP),
                "wo2": np.ascontiguousarray(wo_c[128:]).astype(BFNP),
            }
        )
    return in_maps


def run(in_maps, trace=False, **kwargs):
    nc = get_nc()
    return run_bass_kernel_spmd(
        nc, in_maps, core_ids=list(range(NCORES)), trace=trace, **kwargs
    )


def assemble(results):
    """Gather per-core results into (out, attn) full outputs."""
    out = np.zeros((B, N, DIM), np.float32)
    attn = np.empty((B, HEADS, N, N), np.float32)
    for c in range(NCORES):
        b = c // CPB
        h0 = HPC * (c % CPB)
        res = results[c]
        out[b] += res["out_part"].reshape(N, DIM)
        # attn_t is [h, k, q]; reference layout is [q, k]
        attn[b, h0 : h0 + HPC] = res["attn_t"].transpose(0, 2, 1)
    return out, attn


def kernel(x, labels, con, gamma, beta, w_qkv, w_out):
    in_maps = make_in_maps(x, labels, con, gamma, beta, w_qkv, w_out)
    results = run(in_maps).results
    return assemble(results)


# revision 23
# speedup vs baseline: 1.0138x; 1.0138x over previous
"""Trainium2 Bass kernel for masked+modulated multi-head attention.

Reference computation (per batch b):
    xn = LayerNorm(x) * gamma + beta
    q,k,v = split(xn @ w_qkv); per head: dots = (q k^T) * scale
    dots = where(labels==0 on key, -1e9, dots) * (1 + con)
    attn = softmax(dots, axis=key);  out = (attn @ v reshaped) @ w_out
    returns (out, attn)

Sharding: 8 cores = 2 batches x 4 head-groups (3 heads each).
Each core computes, entirely on device, for its (b, 3 heads):
  - LayerNorm of x[b] (gamma/beta folded into w_qkv / bias on host)
  - q^T,k^T,v per head with an augmented contraction row that applies the
    key padding mask inside the dots matmul (q row 64 = 1, k row 64 = mask)
  - scores TRANSPOSED: S^T[k,q] tiles so that softmax numerators p=exp(s)
    feed the attn@v matmul directly (no on-chip transpose of the 50MB attn)
  - softmax denominators via ones-columns appended to v: PSUM rows 64..127
    of the attn@v output all hold sum_k exp(s), so the reciprocal runs
    64 lanes wide and no cross-partition broadcast is needed
  - attn^T (written k-major; host views it back) and the w_out partial sum
Matmul operands are bf16 (f32 accumulation in PSUM); softmax s stays f32.
k-tiles are processed in pairs so DVE/ACT ops run at [128,1024] granularity.
Host: shards inputs, transposes con once, sums the 4 out-partials per batch,
and transposes attn back to [b, h, q, k] (numpy view manipulation only).
"""

import numpy as np
import ml_dtypes

import concourse.bass as bass
import concourse.bacc as baccmod
import concourse.mybir as mybir
import concourse.tile as tile
from concourse.bass_utils import run_bass_kernel_spmd
from concourse.masks import make_identity

HEADS = 12
DH = 64
DIM = 768
N = 2048
B = 2
NCORES = 8
HPC = 3          # heads per core
CPB = 4          # cores per batch
SCALE = DH ** -0.5
EPS = 1e-5
NEG = -1e9

F32 = mybir.dt.float32
BF16 = mybir.dt.bfloat16
AF = mybir.ActivationFunctionType
ALU = mybir.AluOpType
BFNP = ml_dtypes.bfloat16

NT = N // 128            # 16 k token tiles
NP = NT // 2             # 8 k tile pairs
NQ = N // 512            # 4 query chunks
KF = DIM // 128          # 6 feature k-tiles

# which engine runs the attn normalize+store multiply, per k-pair index
NORM_ENGINE = ["gpsimd", "vector"] * (NP // 2)


def build_nc() -> bass.Bass:
    nc = baccmod.Bacc("TRN2")

    xb = nc.declare_dram_parameter("xb", [NT, 128, DIM], F32, isOutput=False)
    conT = nc.declare_dram_parameter("conT", [N, N], F32, isOutput=False)
    wqkv = nc.declare_dram_parameter("wqkv", [KF, 128, 576], BF16, isOutput=False)
    bqkv = nc.declare_dram_parameter("bqkv", [5, 128], F32, isOutput=False)
    maskrow = nc.declare_dram_parameter("maskrow", [1, N], BF16, isOutput=False)
    wo01 = nc.declare_dram_parameter("wo01", [128, DIM], BF16, isOutput=False)
    wo2 = nc.declare_dram_parameter("wo2", [64, DIM], BF16, isOutput=False)
    attn_t = nc.declare_dram_parameter("attn_t", [HPC, N, N], F32, isOutput=True)
    out_part = nc.declare_dram_parameter("out_part", [NT, 128, DIM], F32, isOutput=True)

    with tile.TileContext(nc) as tc:
        with (
            tc.tile_pool(name="singles", bufs=1) as singles,
            tc.tile_pool(name="persist", bufs=1) as persist,
        ):
            identb = singles.tile([128, 128], BF16, tag="identb")
            make_identity(nc, identb)
            eps_sb = singles.tile([128, 1], F32, tag="eps")
            nc.vector.memset(eps_sb, EPS)
            wo01_sb = singles.tile([128, DIM], BF16, tag="wo01")
            nc.sync.dma_start(out=wo01_sb, in_=wo01[:])
            wo2_sb = singles.tile([64, DIM], BF16, tag="wo2")
            nc.sync.dma_start(out=wo2_sb, in_=wo2[:])

            # persistent per-head tensors (all bf16 matmul operands)
            qT = [persist.tile([65, N], BF16, tag=f"qT{h}", name=f"qT{h}")
                  for h in range(HPC)]
            kT = [persist.tile([65, N], BF16, tag=f"kT{h}", name=f"kT{h}")
                  for h in range(HPC)]
            # v2 cols 0..63 = v^T; cols 64..127 all ones (softmax denominators)
            v2 = [persist.tile([128, NT, 128], BF16, tag=f"v2{h}", name=f"v2{h}")
                  for h in range(HPC)]
            O01 = persist.tile([128, N], BF16, tag="O01")
            O2 = persist.tile([64, N], BF16, tag="O2")

            for h in range(HPC):
                nc.vector.memset(qT[h][64:65, :], 1.0)
                nc.sync.dma_start(out=kT[h][64:65, :], in_=maskrow[:])
                nc.vector.memset(v2[h], 1.0)  # cols 0..63 overwritten later

            # conp pool created BEFORE phase A pools so its SBUF range is
            # disjoint: the first query chunks' con loads prefetch during
            # phase A instead of waiting for phase A buffers to free
            _pconp_cm = tc.tile_pool(name="conp", bufs=2)
            pconp = _pconp_cm.__enter__()

            # ---------------- Phase A: LN + transpose + QKV ----------------
            with (
                tc.tile_pool(name="pa", bufs=3) as pa,
                tc.tile_pool(name="xt", bufs=1) as xtp,
                tc.tile_pool(name="pa_ps", bufs=2, space="PSUM") as pap,
            ):
                w_sb = [xtp.tile([128, 576], BF16, tag=f"w{i}", name=f"w{i}")
                        for i in range(KF)]
                for i in range(KF):
                    nc.sync.dma_start(out=w_sb[i], in_=wqkv[i])
                bias_sb = xtp.tile([128, 5], F32, tag="bias")
                for g in range(5):
                    nc.sync.dma_start(
                        out=bias_sb[:, g : g + 1], in_=bqkv[g].unsqueeze(1)
                    )
                xhatT = [xtp.tile([128, N], BF16, tag=f"xt{i}", name=f"xt{i}")
                         for i in range(KF)]
                vt_tmp = [xtp.tile([64, N], BF16, tag=f"vt{h}", name=f"vt{h}")
                          for h in range(HPC)]

                for tt in range(NT):
                    xtile = pa.tile([128, DIM], F32, tag="x")
                    nc.sync.dma_start(out=xtile, in_=xb[tt])
                    stats = pa.tile([128, 2, 6], F32, tag="st")
                    nc.vector.bn_stats(out=stats[:, 0, :], in_=xtile[:, 0:512])
                    nc.vector.bn_stats(out=stats[:, 1, :], in_=xtile[:, 512:768])
                    mv = pa.tile([128, 2], F32, tag="mv")
                    nc.vector.bn_aggr(out=mv, in_=stats)
                    rstd = pa.tile([128, 1], F32, tag="rstd")
                    nc.scalar.activation(
                        out=rstd, in_=mv[:, 1:2], func=AF.Sqrt, bias=eps_sb
                    )
                    nc.vector.reciprocal(out=rstd, in_=rstd)
                    nmu = pa.tile([128, 1], F32, tag="nmu")
                    nc.vector.scalar_tensor_tensor(
                        out=nmu, in0=mv[:, 0:1], scalar=-1.0, in1=rstd,
                        op0=ALU.mult, op1=ALU.mult,
                    )
                    xhat = pa.tile([128, DIM], BF16, tag="xh")
                    nc.scalar.activation(
                        out=xhat, in_=xtile, func=AF.Identity, bias=nmu, scale=rstd
                    )
                    for ft in range(KF):
                        pst = pap.tile([128, 128], BF16, tag="tr")
                        nc.tensor.transpose(
                            pst, xhat[:, ft * 128 : (ft + 1) * 128], identb
                        )
                        nc.vector.tensor_copy(
                            out=xhatT[ft][:, tt * 128 : (tt + 1) * 128], in_=pst
                        )

                # QKV projection, head-major column order:
                # [q0 k0 v0 q1 k1 v1 q2 k2 v2] (64 each) so head 0's
                # attention can start while later heads still project
                groups = [(0, 128), (128, 256), (256, 384), (384, 512), (512, 576)]
                destmap = [
                    [qT[0], kT[0]],
                    [vt_tmp[0], qT[1]],
                    [kT[1], vt_tmp[1]],
                    [qT[2], kT[2]],
                    [vt_tmp[2]],
                ]
                vdone = {1: 0, 2: 1, 4: 2}  # group -> head whose v completes

                def v_transpose(h):
                    for kt in range(NT):
                        psv = pap.tile([128, 64], BF16, tag="trv", name="psv")
                        nc.tensor.transpose(
                            psv,
                            vt_tmp[h][0:64, kt * 128 : (kt + 1) * 128],
                            identb[0:64, 0:64],
                        )
                        nc.scalar.copy(out=v2[h][:, kt, 0:64], in_=psv)

                for g, (c0, c1) in enumerate(groups):
                    m = c1 - c0
                    for t4 in range(NQ):
                        ps = pap.tile([128, 512], F32, tag="mm")
                        for kc in range(KF):
                            nc.tensor.matmul(
                                ps[:m],
                                w_sb[kc][:, c0:c1],
                                xhatT[kc][:, t4 * 512 : (t4 + 1) * 512],
                                start=(kc == 0),
                                stop=(kc == KF - 1),
                            )
                        for half, dest in enumerate(destmap[g]):
                            nc.scalar.activation(
                                out=dest[0:64, t4 * 512 : (t4 + 1) * 512],
                                in_=ps[half * 64 : half * 64 + 64],
                                func=AF.Identity,
                                bias=bias_sb[half * 64 : half * 64 + 64, g : g + 1],
                            )
                    if g in vdone:
                        v_transpose(vdone[g])

            # ---------------- Phase B: attention ----------------
            with (
                tc.tile_pool(name="ps_s", bufs=2) as pss,
                tc.tile_pool(name="pp", bufs=2) as pp,
                tc.tile_pool(name="pat", bufs=2) as pat,
                tc.tile_pool(name="psmall", bufs=2) as psmall,
                tc.tile_pool(name="pcc", bufs=2) as pcc,
                tc.tile_pool(name="pb_s", bufs=2, space="PSUM") as pbs,
                tc.tile_pool(name="pb_o", bufs=2, space="PSUM") as pbo,
            ):
                def emit_av(h, kp, psO, pkp):
                    for j in range(2):
                        kt = 2 * kp + j
                        nc.tensor.matmul(
                            psO,
                            v2[h][:, kt, :],
                            pkp[:, j, :],
                            start=(kt == 0),
                            stop=(kt == NT - 1),
                        )

                # software pipeline across (qc, h) blocks: block i's attn
                # normalize+store multiplies are interleaved into block i+1's
                # score loop so DVE never runs a long norm burst that starves
                # the next head's psS slots
                prev = None  # (pk_pairs, rb, atb, h, q0)

                def norm_step(kp):
                    pk_pairs, rb, atb, ph, pq0 = prev
                    nc.vector.tensor_tensor(
                        atb[:, 2 * kp : 2 * kp + 2, :],
                        pk_pairs[kp],
                        rb.unsqueeze(1).broadcast_to([128, 2, 512]),
                        ALU.mult,
                    )

                def norm_flush():
                    _, _, atb, ph, pq0 = prev
                    nc.gpsimd.dma_start(
                        out=attn_t[ph].rearrange(
                            "(kt p) q -> p kt q", p=128
                        )[:, :, pq0 : pq0 + 512],
                        in_=atb,
                    )

                for qc in range(NQ):
                    q0 = qc * 512
                    conp = pconp.tile([128, NT, 512], F32, tag="conp")
                    for kt in range(NT):
                        nc.sync.dma_start(
                            out=conp[:, kt, :],
                            in_=conT[kt * 128 : (kt + 1) * 128, q0 : q0 + 512],
                        )
                    for kp in range(NP):
                        nc.vector.tensor_scalar_add(
                            conp[:, 2 * kp : 2 * kp + 2, :],
                            conp[:, 2 * kp : 2 * kp + 2, :],
                            1.0,
                        )
                    for h in range(HPC):
                        psO = pbo.tile([128, 512], F32, tag="O")
                        pk_pairs = []
                        for kp in range(NP):
                            psSp = pbs.tile([128, 2, 512], F32, tag="S")
                            for j in range(2):
                                kt = 2 * kp + j
                                nc.tensor.matmul(
                                    psSp[:, j, :],
                                    kT[h][:, kt * 128 : (kt + 1) * 128],
                                    qT[h][:, q0 : q0 + 512],
                                    start=True,
                                    stop=True,
                                )
                            sp = pss.tile([128, 2, 512], F32, tag="s")
                            nc.vector.tensor_mul(
                                sp, psSp, conp[:, 2 * kp : 2 * kp + 2, :]
                            )
                            pkp = pp.tile([128, 2, 512], BF16, tag=f"p{kp}",
                                          name=f"p{kp}")
                            nc.scalar.activation(out=pkp, in_=sp, func=AF.Exp)
                            pk_pairs.append(pkp)
                            # AV matmuls lag one pair so the tensor engine
                            # never stalls on the freshest exp
                            if kp >= 1:
                                emit_av(h, kp - 1, psO, pk_pairs[kp - 1])
                            if prev is not None:
                                norm_step(kp)
                        emit_av(h, NP - 1, psO, pk_pairs[NP - 1])
                        if prev is not None:
                            norm_flush()
                        # row 64 of psO holds sum_k exp(s) per q
                        sums = psmall.tile([1, 512], F32, tag="sums")
                        nc.vector.tensor_copy(out=sums, in_=psO[64:65, :])
                        recip = psmall.tile([1, 512], F32, tag="recip")
                        nc.vector.reciprocal_approx_fast(out=recip, in_=sums)
                        recipb = psmall.tile([1, 512], BF16, tag="recipb")
                        nc.vector.tensor_copy(out=recipb, in_=recip)
                        rb = psmall.tile([128, 512], BF16, tag="rb")
                        nc.gpsimd.partition_broadcast(rb, recipb)
                        atb = pat.tile([128, NT, 512], BF16, tag="at")
                        if h < 2:
                            dest = O01[h * 64 : (h + 1) * 64, q0 : q0 + 512]
                        else:
                            dest = O2[:, q0 : q0 + 512]
                        nc.vector.tensor_mul(dest, psO[0:64, :], rb[0:64, :])
                        prev = (pk_pairs, rb, atb, h, q0)
                    # output projection for this qc's token tiles (O columns
                    # for q0..q0+512 are final once all 3 heads are done)
                    for tt in range(4 * qc, 4 * qc + 4):
                        outp = pcc.tile([128, DIM], F32, tag="op")
                        for n0, n1 in [(0, 512), (512, 768)]:
                            pso = pbo.tile([128, 512], F32, tag="mm2")
                            nc.tensor.matmul(
                                pso[:, : n1 - n0],
                                O01[:, tt * 128 : (tt + 1) * 128],
                                wo01_sb[:, n0:n1],
                                start=True,
                                stop=False,
                            )
                            nc.tensor.matmul(
                                pso[:, : n1 - n0],
                                O2[:, tt * 128 : (tt + 1) * 128],
                                wo2_sb[:, n0:n1],
                                start=False,
                                stop=True,
                            )
                            nc.scalar.copy(out=outp[:, n0:n1], in_=pso[:, : n1 - n0])
                        nc.sync.dma_start(out=out_part[tt], in_=outp)
                # epilogue: normalize+store the final block
                for kp in range(NP):
                    norm_step(kp)
                norm_flush()

            _pconp_cm.__exit__(None, None, None)

    nc.compile()
    return nc


_NC_CACHE = None


def get_nc() -> bass.Bass:
    global _NC_CACHE
    if _NC_CACHE is None:
        _NC_CACHE = build_nc()
    return _NC_CACHE


def make_in_maps(x, labels, con, gamma, beta, w_qkv, w_out):
    """Host-side sharding: returns list of 8 per-core input dicts."""
    x = np.asarray(x, dtype=np.float32)
    labels = np.asarray(labels)
    con = np.asarray(con, dtype=np.float32)
    gamma = np.asarray(gamma, dtype=np.float32)
    beta = np.asarray(beta, dtype=np.float32)
    w_qkv = np.asarray(w_qkv, dtype=np.float32)
    w_out = np.asarray(w_out, dtype=np.float32)

    # fold gamma into the qkv weights, and q's 1/sqrt(d) scale into wq/bq
    wq = (gamma[:, None] * w_qkv[:, :DIM]) * SCALE
    wk = gamma[:, None] * w_qkv[:, DIM : 2 * DIM]
    wv = gamma[:, None] * w_qkv[:, 2 * DIM :]
    bq = (beta @ w_qkv[:, :DIM]) * SCALE
    bk = beta @ w_qkv[:, DIM : 2 * DIM]
    bv = beta @ w_qkv[:, 2 * DIM :]

    in_maps = []
    for c in range(NCORES):
        b = c // CPB
        h0 = HPC * (c % CPB)
        cols = []
        bcols = []
        for h in range(HPC):
            sl = slice((h0 + h) * DH, (h0 + h + 1) * DH)
            cols += [wq[:, sl], wk[:, sl], wv[:, sl]]
            bcols += [bq[sl], bk[sl], bv[sl]]
        wqkv_c = np.concatenate(cols, axis=1)  # [768, 576]
        bqkv_c = np.concatenate(bcols + [np.zeros(64, np.float32)]).reshape(5, 128)
        wo_c = w_out[h0 * DH : (h0 + HPC) * DH]  # [192, 768]
        in_maps.append(
            {
                "xb": np.ascontiguousarray(x[b].reshape(NT, 128, DIM)),
                "conT": np.ascontiguousarray(con[b].T),
                "wqkv": np.ascontiguousarray(wqkv_c.reshape(KF, 128, 576)).astype(BFNP),
                "bqkv": np.ascontiguousarray(bqkv_c),
                "maskrow": np.where(labels[b] == 0, np.float32(NEG), np.float32(0.0))
                .astype(BFNP)
                .reshape(1, N),
                "wo01": np.ascontiguousarray(wo_c[:128]).astype(BFNP),
                "wo2": np.ascontiguousarray(wo_c[128:]).astype(BFNP),
            }
        )
    return in_maps


def run(in_maps, trace=False, **kwargs):
    nc = get_nc()
    return run_bass_kernel_spmd(
        nc, in_maps, core_ids=list(range(NCORES)), trace=trace, **kwargs
    )


def assemble(results):
    """Gather per-core results into (out, attn) full outputs."""
    out = np.zeros((B, N, DIM), np.float32)
    attn = np.empty((B, HEADS, N, N), np.float32)
    for c in range(NCORES):
        b = c // CPB
        h0 = HPC * (c % CPB)
        res = results[c]
        out[b] += res["out_part"].reshape(N, DIM)
        # attn_t is [h, k, q]; reference layout is [q, k]
        attn[b, h0 : h0 + HPC] = res["attn_t"].transpose(0, 2, 1)
    return out, attn


def kernel(x, labels, con, gamma, beta, w_qkv, w_out):
    in_maps = make_in_maps(x, labels, con, gamma, beta, w_qkv, w_out)
    results = run(in_maps).results
    return assemble(results)


# revision 24
# speedup vs baseline: 1.0599x; 1.0454x over previous
"""Trainium2 Bass kernel for masked+modulated multi-head attention.

Reference computation (per batch b):
    xn = LayerNorm(x) * gamma + beta
    q,k,v = split(xn @ w_qkv); per head: dots = (q k^T) * scale
    dots = where(labels==0 on key, -1e9, dots) * (1 + con)
    attn = softmax(dots, axis=key);  out = (attn @ v reshaped) @ w_out
    returns (out, attn)

Sharding: 8 cores = 2 batches x 4 head-groups (3 heads each).
Each core computes, entirely on device, for its (b, 3 heads):
  - LayerNorm of x[b] (gamma/beta folded into w_qkv / bias on host)
  - q^T,k^T,v per head with an augmented contraction row that applies the
    key padding mask inside the dots matmul (q row 64 = 1, k row 64 = mask)
  - scores TRANSPOSED: S^T[k,q] tiles so that softmax numerators p=exp(s)
    feed the attn@v matmul directly (no on-chip transpose of the 50MB attn)
  - softmax denominators via ones-columns appended to v: PSUM rows 64..127
    of the attn@v output all hold sum_k exp(s), so the reciprocal runs
    64 lanes wide and no cross-partition broadcast is needed
  - attn^T (written k-major; host views it back) and the w_out partial sum
Matmul operands are bf16 (f32 accumulation in PSUM); softmax s stays f32.
k-tiles are processed in pairs so DVE/ACT ops run at [128,1024] granularity.
Host: shards inputs, transposes con once, sums the 4 out-partials per batch,
and transposes attn back to [b, h, q, k] (numpy view manipulation only).
"""

import numpy as np
import ml_dtypes

import concourse.bass as bass
import concourse.bacc as baccmod
import concourse.mybir as mybir
import concourse.tile as tile
from concourse.bass_utils import run_bass_kernel_spmd
from concourse.masks import make_identity

HEADS = 12
DH = 64
DIM = 768
N = 2048
B = 2
NCORES = 8
HPC = 3          # heads per core
CPB = 4          # cores per batch
SCALE = DH ** -0.5
EPS = 1e-5
NEG = -1e9

F32 = mybir.dt.float32
BF16 = mybir.dt.bfloat16
AF = mybir.ActivationFunctionType
ALU = mybir.AluOpType
BFNP = ml_dtypes.bfloat16

NT = N // 128            # 16 k token tiles
NP = NT // 2             # 8 k tile pairs
NQ = N // 512            # 4 query chunks
KF = DIM // 128          # 6 feature k-tiles

# which engine runs the attn normalize+store multiply, per k-pair index
NORM_ENGINE = ["gpsimd", "vector"] * (NP // 2)


def build_nc() -> bass.Bass:
    nc = baccmod.Bacc("TRN2")

    xb = nc.declare_dram_parameter("xb", [NT, 128, DIM], F32, isOutput=False)
    conT = nc.declare_dram_parameter("conT", [N, N], F32, isOutput=False)
    wqkv = nc.declare_dram_parameter("wqkv", [KF, 128, 576], BF16, isOutput=False)
    bqkv = nc.declare_dram_parameter("bqkv", [5, 128], F32, isOutput=False)
    maskrow = nc.declare_dram_parameter("maskrow", [1, N], BF16, isOutput=False)
    wo01 = nc.declare_dram_parameter("wo01", [128, DIM], BF16, isOutput=False)
    wo2 = nc.declare_dram_parameter("wo2", [64, DIM], BF16, isOutput=False)
    attn_t = nc.declare_dram_parameter("attn_t", [HPC, N, N], F32, isOutput=True)
    out_part = nc.declare_dram_parameter("out_part", [NT, 128, DIM], F32, isOutput=True)

    with tile.TileContext(nc) as tc:
        with (
            tc.tile_pool(name="singles", bufs=1) as singles,
            tc.tile_pool(name="persist", bufs=1) as persist,
        ):
            identb = singles.tile([128, 128], BF16, tag="identb")
            make_identity(nc, identb)
            eps_sb = singles.tile([128, 1], F32, tag="eps")
            nc.vector.memset(eps_sb, EPS)
            wo01_sb = singles.tile([128, DIM], BF16, tag="wo01")
            nc.sync.dma_start(out=wo01_sb, in_=wo01[:])
            wo2_sb = singles.tile([64, DIM], BF16, tag="wo2")
            nc.sync.dma_start(out=wo2_sb, in_=wo2[:])

            # persistent per-head tensors (all bf16 matmul operands)
            qT = [persist.tile([65, N], BF16, tag=f"qT{h}", name=f"qT{h}")
                  for h in range(HPC)]
            kT = [persist.tile([65, N], BF16, tag=f"kT{h}", name=f"kT{h}")
                  for h in range(HPC)]
            # v2 cols 0..63 = v^T; cols 64..127 all ones (softmax denominators)
            v2 = [persist.tile([128, NT, 128], BF16, tag=f"v2{h}", name=f"v2{h}")
                  for h in range(HPC)]
            O01 = persist.tile([128, N], BF16, tag="O01")
            O2 = persist.tile([64, N], BF16, tag="O2")

            for h in range(HPC):
                nc.vector.memset(qT[h][64:65, :], 1.0)
                nc.sync.dma_start(out=kT[h][64:65, :], in_=maskrow[:])
                nc.vector.memset(v2[h], 1.0)  # cols 0..63 overwritten later

            # conp pool created BEFORE phase A pools so its SBUF range is
            # disjoint: the first query chunks' con loads prefetch during
            # phase A instead of waiting for phase A buffers to free
            _pconp_cm = tc.tile_pool(name="conp", bufs=2)
            pconp = _pconp_cm.__enter__()

            # ---------------- Phase A: LN + transpose + QKV ----------------
            with (
                tc.tile_pool(name="pa", bufs=3) as pa,
                tc.tile_pool(name="xt", bufs=1) as xtp,
                tc.tile_pool(name="pa_ps", bufs=2, space="PSUM") as pap,
            ):
                w_sb = [xtp.tile([128, 576], BF16, tag=f"w{i}", name=f"w{i}")
                        for i in range(KF)]
                for i in range(KF):
                    nc.sync.dma_start(out=w_sb[i], in_=wqkv[i])
                bias_sb = xtp.tile([128, 5], F32, tag="bias")
                for g in range(5):
                    nc.sync.dma_start(
                        out=bias_sb[:, g : g + 1], in_=bqkv[g].unsqueeze(1)
                    )
                xhatT = [xtp.tile([128, N], BF16, tag=f"xt{i}", name=f"xt{i}")
                         for i in range(KF)]
                vt_tmp = [xtp.tile([64, N], BF16, tag=f"vt{h}", name=f"vt{h}")
                          for h in range(HPC)]

                # QKV column order is head-major: [q0 k0 v0 q1 k1 v1 q2 k2 v2]
                groups = [(0, 128), (128, 256), (256, 384), (384, 512), (512, 576)]
                destmap = [
                    [qT[0], kT[0]],
                    [vt_tmp[0], qT[1]],
                    [kT[1], vt_tmp[1]],
                    [qT[2], kT[2]],
                    [vt_tmp[2]],
                ]

                # LN + transpose for one token tile
                def ln_tile(tt):
                    xtile = pa.tile([128, DIM], F32, tag="x", name="xtile")
                    nc.sync.dma_start(out=xtile, in_=xb[tt])
                    stats = pa.tile([128, 2, 6], F32, tag="st", name="stats")
                    nc.vector.bn_stats(out=stats[:, 0, :], in_=xtile[:, 0:512])
                    nc.vector.bn_stats(out=stats[:, 1, :], in_=xtile[:, 512:768])
                    mv = pa.tile([128, 2], F32, tag="mv", name="mv")
                    nc.vector.bn_aggr(out=mv, in_=stats)
                    rstd = pa.tile([128, 1], F32, tag="rstd", name="rstd")
                    nc.scalar.activation(
                        out=rstd, in_=mv[:, 1:2], func=AF.Sqrt, bias=eps_sb
                    )
                    nc.vector.reciprocal(out=rstd, in_=rstd)
                    nmu = pa.tile([128, 1], F32, tag="nmu", name="nmu")
                    nc.vector.scalar_tensor_tensor(
                        out=nmu, in0=mv[:, 0:1], scalar=-1.0, in1=rstd,
                        op0=ALU.mult, op1=ALU.mult,
                    )
                    xhat = pa.tile([128, DIM], BF16, tag="xh", name="xhat")
                    nc.scalar.activation(
                        out=xhat, in_=xtile, func=AF.Identity, bias=nmu, scale=rstd
                    )
                    for ft in range(KF):
                        pst = pap.tile([128, 128], BF16, tag="tr", name="pst")
                        nc.tensor.transpose(
                            pst, xhat[:, ft * 128 : (ft + 1) * 128], identb
                        )
                        nc.vector.tensor_copy(
                            out=xhatT[ft][:, tt * 128 : (tt + 1) * 128], in_=pst
                        )

                def qkv_chunk(g, t4):
                    c0, c1 = groups[g]
                    m = c1 - c0
                    ps = pap.tile([128, 512], F32, tag="mm", name="ps")
                    for kc in range(KF):
                        nc.tensor.matmul(
                            ps[:m],
                            w_sb[kc][:, c0:c1],
                            xhatT[kc][:, t4 * 512 : (t4 + 1) * 512],
                            start=(kc == 0),
                            stop=(kc == KF - 1),
                        )
                    for half, dest in enumerate(destmap[g]):
                        nc.scalar.activation(
                            out=dest[0:64, t4 * 512 : (t4 + 1) * 512],
                            in_=ps[half * 64 : half * 64 + 64],
                            func=AF.Identity,
                            bias=bias_sb[half * 64 : half * 64 + 64, g : g + 1],
                        )

                def v_transpose(h):
                    for kt in range(NT):
                        psv = pap.tile([128, 64], BF16, tag="trv", name="psv")
                        nc.tensor.transpose(
                            psv,
                            vt_tmp[h][0:64, kt * 128 : (kt + 1) * 128],
                            identb[0:64, 0:64],
                        )
                        nc.scalar.copy(out=v2[h][:, kt, 0:64], in_=psv)

                # interleave: LN a 4-tile token chunk, then project it for all
                # groups — keeps PE fed while the next chunk's LN runs
                for t4 in range(NQ):
                    for tt in range(4 * t4, 4 * t4 + 4):
                        ln_tile(tt)
                    for g in range(5):
                        qkv_chunk(g, t4)
                for h in range(HPC):
                    v_transpose(h)

            # ---------------- Phase B: attention ----------------
            with (
                tc.tile_pool(name="ps_s", bufs=2) as pss,
                tc.tile_pool(name="pp", bufs=2) as pp,
                tc.tile_pool(name="pat", bufs=2) as pat,
                tc.tile_pool(name="psmall", bufs=2) as psmall,
                tc.tile_pool(name="pcc", bufs=2) as pcc,
                tc.tile_pool(name="pb_s", bufs=2, space="PSUM") as pbs,
                tc.tile_pool(name="pb_o", bufs=2, space="PSUM") as pbo,
            ):
                def emit_av(h, kp, psO, pkp):
                    for j in range(2):
                        kt = 2 * kp + j
                        nc.tensor.matmul(
                            psO,
                            v2[h][:, kt, :],
                            pkp[:, j, :],
                            start=(kt == 0),
                            stop=(kt == NT - 1),
                        )

                # software pipeline across (qc, h) blocks: block i's attn
                # normalize+store multiplies are interleaved into block i+1's
                # score loop so DVE never runs a long norm burst that starves
                # the next head's psS slots
                prev = None  # (pk_pairs, rb, atb, h, q0)

                def norm_step(kp):
                    pk_pairs, rb, atb, ph, pq0 = prev
                    nc.vector.tensor_tensor(
                        atb[:, 2 * kp : 2 * kp + 2, :],
                        pk_pairs[kp],
                        rb.unsqueeze(1).broadcast_to([128, 2, 512]),
                        ALU.mult,
                    )

                def norm_flush():
                    _, _, atb, ph, pq0 = prev
                    nc.gpsimd.dma_start(
                        out=attn_t[ph].rearrange(
                            "(kt p) q -> p kt q", p=128
                        )[:, :, pq0 : pq0 + 512],
                        in_=atb,
                    )

                for qc in range(NQ):
                    q0 = qc * 512
                    conp = pconp.tile([128, NT, 512], F32, tag="conp")
                    for kt in range(NT):
                        nc.sync.dma_start(
                            out=conp[:, kt, :],
                            in_=conT[kt * 128 : (kt + 1) * 128, q0 : q0 + 512],
                        )
                    for kp in range(NP):
                        nc.vector.tensor_scalar_add(
                            conp[:, 2 * kp : 2 * kp + 2, :],
                            conp[:, 2 * kp : 2 * kp + 2, :],
                            1.0,
                        )
                    for h in range(HPC):
                        psO = pbo.tile([128, 512], F32, tag="O")
                        pk_pairs = []
                        for kp in range(NP):
                            psSp = pbs.tile([128, 2, 512], F32, tag="S")
                            for j in range(2):
                                kt = 2 * kp + j
                                nc.tensor.matmul(
                                    psSp[:, j, :],
                                    kT[h][:, kt * 128 : (kt + 1) * 128],
                                    qT[h][:, q0 : q0 + 512],
                                    start=True,
                                    stop=True,
                                )
                            sp = pss.tile([128, 2, 512], F32, tag="s")
                            nc.vector.tensor_mul(
                                sp, psSp, conp[:, 2 * kp : 2 * kp + 2, :]
                            )
                            pkp = pp.tile([128, 2, 512], BF16, tag=f"p{kp}",
                                          name=f"p{kp}")
                            nc.scalar.activation(out=pkp, in_=sp, func=AF.Exp)
                            pk_pairs.append(pkp)
                            # AV matmuls lag one pair so the tensor engine
                            # never stalls on the freshest exp
                            if kp >= 1:
                                emit_av(h, kp - 1, psO, pk_pairs[kp - 1])
                            if prev is not None:
                                norm_step(kp)
                        emit_av(h, NP - 1, psO, pk_pairs[NP - 1])
                        if prev is not None:
                            norm_flush()
                        # row 64 of psO holds sum_k exp(s) per q
                        sums = psmall.tile([1, 512], F32, tag="sums")
                        nc.vector.tensor_copy(out=sums, in_=psO[64:65, :])
                        recip = psmall.tile([1, 512], F32, tag="recip")
                        nc.vector.reciprocal_approx_fast(out=recip, in_=sums)
                        recipb = psmall.tile([1, 512], BF16, tag="recipb")
                        nc.vector.tensor_copy(out=recipb, in_=recip)
                        rb = psmall.tile([128, 512], BF16, tag="rb")
                        nc.gpsimd.partition_broadcast(rb, recipb)
                        atb = pat.tile([128, NT, 512], BF16, tag="at")
                        if h < 2:
                            dest = O01[h * 64 : (h + 1) * 64, q0 : q0 + 512]
                        else:
                            dest = O2[:, q0 : q0 + 512]
                        nc.vector.tensor_mul(dest, psO[0:64, :], rb[0:64, :])
                        prev = (pk_pairs, rb, atb, h, q0)
                    # output projection for this qc's token tiles (O columns
                    # for q0..q0+512 are final once all 3 heads are done)
                    for tt in range(4 * qc, 4 * qc + 4):
                        outp = pcc.tile([128, DIM], F32, tag="op")
                        for n0, n1 in [(0, 512), (512, 768)]:
                            pso = pbo.tile([128, 512], F32, tag="mm2")
                            nc.tensor.matmul(
                                pso[:, : n1 - n0],
                                O01[:, tt * 128 : (tt + 1) * 128],
                                wo01_sb[:, n0:n1],
                                start=True,
                                stop=False,
                            )
                            nc.tensor.matmul(
                                pso[:, : n1 - n0],
                                O2[:, tt * 128 : (tt + 1) * 128],
                                wo2_sb[:, n0:n1],
                                start=False,
                                stop=True,
                            )
                            nc.scalar.copy(out=outp[:, n0:n1], in_=pso[:, : n1 - n0])
                        nc.sync.dma_start(out=out_part[tt], in_=outp)
                # epilogue: normalize+store the final block
                for kp in range(NP):
                    norm_step(kp)
                norm_flush()

            _pconp_cm.__exit__(None, None, None)

    nc.compile()
    return nc


_NC_CACHE = None


def get_nc() -> bass.Bass:
    global _NC_CACHE
    if _NC_CACHE is None:
        _NC_CACHE = build_nc()
    return _NC_CACHE


def make_in_maps(x, labels, con, gamma, beta, w_qkv, w_out):
    """Host-side sharding: returns list of 8 per-core input dicts."""
    x = np.asarray(x, dtype=np.float32)
    labels = np.asarray(labels)
    con = np.asarray(con, dtype=np.float32)
    gamma = np.asarray(gamma, dtype=np.float32)
    beta = np.asarray(beta, dtype=np.float32)
    w_qkv = np.asarray(w_qkv, dtype=np.float32)
    w_out = np.asarray(w_out, dtype=np.float32)

    # fold gamma into the qkv weights, and q's 1/sqrt(d) scale into wq/bq
    wq = (gamma[:, None] * w_qkv[:, :DIM]) * SCALE
    wk = gamma[:, None] * w_qkv[:, DIM : 2 * DIM]
    wv = gamma[:, None] * w_qkv[:, 2 * DIM :]
    bq = (beta @ w_qkv[:, :DIM]) * SCALE
    bk = beta @ w_qkv[:, DIM : 2 * DIM]
    bv = beta @ w_qkv[:, 2 * DIM :]

    in_maps = []
    for c in range(NCORES):
        b = c // CPB
        h0 = HPC * (c % CPB)
        cols = []
        bcols = []
        for h in range(HPC):
            sl = slice((h0 + h) * DH, (h0 + h + 1) * DH)
            cols += [wq[:, sl], wk[:, sl], wv[:, sl]]
            bcols += [bq[sl], bk[sl], bv[sl]]
        wqkv_c = np.concatenate(cols, axis=1)  # [768, 576]
        bqkv_c = np.concatenate(bcols + [np.zeros(64, np.float32)]).reshape(5, 128)
        wo_c = w_out[h0 * DH : (h0 + HPC) * DH]  # [192, 768]
        in_maps.append(
            {
                "xb": np.ascontiguousarray(x[b].reshape(NT, 128, DIM)),
                "conT": np.ascontiguousarray(con[b].T),
                "wqkv": np.ascontiguousarray(wqkv_c.reshape(KF, 128, 576)).astype(BFNP),
                "bqkv": np.ascontiguousarray(bqkv_c),
                "maskrow": np.where(labels[b] == 0, np.float32(NEG), np.float32(0.0))
                .astype(BFNP)
                .reshape(1, N),
                "wo01": np.ascontiguousarray(wo_c[:128]).astype(BFNP),
                "wo2": np.ascontiguousarray(wo_c[128:]).astype(BFNP),
            }
        )
    return in_maps


def run(in_maps, trace=False, **kwargs):
    nc = get_nc()
    return run_bass_kernel_spmd(
        nc, in_maps, core_ids=list(range(NCORES)), trace=trace, **kwargs
    )


def assemble(results):
    """Gather per-core results into (out, attn) full outputs."""
    out = np.zeros((B, N, DIM), np.float32)
    attn = np.empty((B, HEADS, N, N), np.float32)
    for c in range(NCORES):
        b = c // CPB
        h0 = HPC * (c % CPB)
        res = results[c]
        out[b] += res["out_part"].reshape(N, DIM)
        # attn_t is [h, k, q]; reference layout is [q, k]
        attn[b, h0 : h0 + HPC] = res["attn_t"].transpose(0, 2, 1)
    return out, attn


def kernel(x, labels, con, gamma, beta, w_qkv, w_out):
    in_maps = make_in_maps(x, labels, con, gamma, beta, w_qkv, w_out)
    results = run(in_maps).results
    return assemble(results)
